# revision 2
# baseline (speedup 1.0000x reference)
"""Trainium2 Bass kernel for nn_DGG_StraightThrough.

The reference's pairwise-logit MLP is mathematically dead: softmax over the
singleton feature dim is identically 1, so log_p == 0 and the gumbel logits
y equal `temp` exactly.  adj[b,i,j] = 1.0 iff temp[i,j] is among the 8
largest of row i, identical across the batch.

Sharding: row-parallel over N=2048 across 8 cores (256 rows/core, two
128-row chunks living side by side in one [128, 4096] SBUF tile).

v2 layout (vs the 23.9us single-queue baseline):
  - Both HWDGE queues stream inputs in parallel: sync carries partitions
    0:64 of each chunk, scalar carries 64:128, all as full-8KB-row
    descriptors (per-queue DGE desc dispatch ~8ns + bytes/340GB/s, so
    big descriptors matter more than piece pipelining).
  - DVE does only the two flat MAX8s; the is_ge mask passes move to
    GpSimd (chunk0, chunk1 col-half b) and DVE (chunk1 col-half a) so
    the threshold compare overlaps MAX8 / the out stream.
  - out0 goes out on the scalar queue while chunk1 still computes; out1
    on the sync queue.

Host: concatenate 8 u8 slabs, cast to f32, broadcast over B=4.
"""

import sys

import numpy as np

if "/opt/trn_rl_repo" not in sys.path:
    sys.path.insert(0, "/opt/trn_rl_repo")

B, N, K = 4, 2048, 8
N_CORES = 8
ROWS = N // N_CORES  # 256 rows per core
P = 128  # SBUF partitions
HP = P // 2  # partition half per queue

# Hooks for a driving harness (test.py): extra kwargs for run_bass_kernel_spmd
# and the last BassKernelResults (exec_time_ns etc).
RUN_KWARGS: dict = {}
LAST_RESULT = None

_PROGRAM = None


def _build_program():
    import concourse.bass as bass
    import concourse.mybir as mybir

    class _LeanBass(bass.Bass):
        # Skip the barrier Bass.__init__ emits after const-AP registration:
        # this kernel never reads const APs, Sync's DGE table load precedes
        # its DMAs in program order, and the NRT entry pseudo-barrier already
        # orders the gpsimd sem-clears.  Saves ~1us of preamble.
        _skip_init_barrier = False

        def all_engine_barrier(self, **kw):
            if _LeanBass._skip_init_barrier:
                return
            return super().all_engine_barrier(**kw)

    _LeanBass._skip_init_barrier = True
    try:
        nc = _LeanBass(enable_partition_id=False, monotonic_sem_count=0)
    finally:
        _LeanBass._skip_init_barrier = False
    t_in = nc.declare_dram_parameter("t", [ROWS, N], mybir.dt.float32, isOutput=False)
    # u8 wire format for the 0/1 mask (lossless); host casts back to f32
    out = nc.declare_dram_parameter("out", [ROWS, N], mybir.dt.uint8, isOutput=True)

    with (
        nc.sbuf_tensor([P, 2 * N], mybir.dt.float32) as tile,
        nc.sbuf_tensor([P, 2 * N], mybir.dt.uint8) as mask,
        nc.sbuf_tensor([P, 16], mybir.dt.float32) as t8,
        # per-transfer in-DMA sems: transfers on different queues complete
        # out of order, so shared counting sems would race
        nc.semaphore("in_a0") as in_a0,
        nc.semaphore("in_b0") as in_b0,
        nc.semaphore("in_a1") as in_a1,
        nc.semaphore("in_b1") as in_b1,
        nc.semaphore("v_sem") as v_sem,
        nc.semaphore("g_sem") as g_sem,
        nc.semaphore("out_sem") as out_sem,
    ):
        # Issue the in-DMAs OUTSIDE the Block, right after each queue
        # engine's DGE-table preamble.  Full 8KB rows = one descriptor per
        # partition; partition-halves split each chunk across both queues.
        nc.sync.dma_start(out=tile[0:HP, 0:N], in_=t_in[0:HP, :]).then_inc(in_a0, 16)
        nc.scalar.dma_start(out=tile[HP:P, 0:N], in_=t_in[HP:P, :]).then_inc(in_b0, 16)
        nc.sync.dma_start(out=tile[0:HP, N : 2 * N], in_=t_in[P : P + HP, :]).then_inc(
            in_a1, 16
        )
        nc.scalar.dma_start(
            out=tile[HP:P, N : 2 * N], in_=t_in[P + HP : 2 * P, :]
        ).then_inc(in_b1, 16)

        # no SWDGE DMAs issued -> skip GpSimd's expensive dge_drain at exit
        with nc.Block(no_gpsimd_drain=True) as block:

            @block.vector
            def _(vector):
                # Flat MAX8 per chunk (top8 incl. the is_ge threshold at
                # [:,7] / [:,15]).  The sem self-hop before the tensor_scalar
                # guards the same-engine RAW on t8 (scalar-ptr fetch races
                # the in-pipeline write of max1).
                vector.wait_ge(in_a0, 16)
                vector.wait_ge(in_b0, 16)
                vector.max(t8[:, 0:8], tile[:, 0:N]).then_inc(v_sem, 1)
                vector.wait_ge(in_a1, 16)
                vector.wait_ge(in_b1, 16)
                vector.max(t8[:, 8:16], tile[:, N : 2 * N]).then_inc(v_sem, 1)
                vector.wait_ge(v_sem, 2)
                vector.tensor_scalar(
                    mask[:, N : N + N // 2],
                    tile[:, N : N + N // 2],
                    t8[:, 15:16],
                    None,
                    mybir.AluOpType.is_ge,
                ).then_inc(v_sem, 1)

            @block.gpsimd
            def _(gpsimd):
                gpsimd.wait_ge(v_sem, 1)
                gpsimd.tensor_scalar(
                    mask[:, 0:N],
                    tile[:, 0:N],
                    t8[:, 7:8],
                    None,
                    mybir.AluOpType.is_ge,
                ).then_inc(g_sem, 1)
                gpsimd.wait_ge(v_sem, 2)
                gpsimd.tensor_scalar(
                    mask[:, N + N // 2 : 2 * N],
                    tile[:, N + N // 2 : 2 * N],
                    t8[:, 15:16],
                    None,
                    mybir.AluOpType.is_ge,
                ).then_inc(g_sem, 1)

            @block.scalar
            def _(scalar):
                # chunk0 mask out on the scalar queue, overlapped with the
                # chunk1 compute
                scalar.wait_ge(g_sem, 1)
                scalar.dma_start(
                    out=out[0:P, :], in_=mask[:, 0:N]
                ).then_inc(out_sem, 16)

            @block.sync
            def _(sync):
                sync.wait_ge(v_sem, 3)
                sync.wait_ge(g_sem, 2)
                sync.dma_start(
                    out=out[P : 2 * P, :], in_=mask[:, N : 2 * N]
                ).then_inc(out_sem, 16)
                sync.wait_ge(out_sem, 32)

    return nc


def kernel(**inputs: np.ndarray) -> np.ndarray:
    global _PROGRAM, LAST_RESULT
    from concourse.bass_utils import run_bass_kernel_spmd

    temp = np.ascontiguousarray(np.asarray(inputs["temp"], dtype=np.float32))
    assert temp.shape == (N, N)

    in_maps = [
        {"t": np.ascontiguousarray(temp[c * ROWS : (c + 1) * ROWS])}
        for c in range(N_CORES)
    ]

    res = None
    last_err = None
    for attempt in range(3):
        try:
            if _PROGRAM is None:
                _PROGRAM = _build_program()
            res = run_bass_kernel_spmd(
                _PROGRAM, in_maps, list(range(N_CORES)), **RUN_KWARGS
            )
            break
        except Exception as e:  # transient device wedges (e.g. NRT unrecoverable)
            last_err = e
            _PROGRAM = None
            if attempt == 2:
                raise
            import time

            time.sleep(10 * (attempt + 1))
            try:  # recreate the PJRT client, as a fresh process would
                import jax

                jax.clear_backends()
                jax.devices()
            except Exception:
                pass
    assert res is not None, last_err
    LAST_RESULT = res

    mask = np.concatenate([res.results[c]["out"] for c in range(N_CORES)], axis=0)
    mask = mask.astype(np.float32)
    return np.ascontiguousarray(np.broadcast_to(mask[None], (B, N, N)))


# revision 4
# speedup vs baseline: 2.5349x; 2.5349x over previous
"""Trainium2 Bass kernel for nn_DGG_StraightThrough.

The reference's pairwise-logit MLP is mathematically dead: softmax over the
singleton feature dim is identically 1, so log_p == 0 and the gumbel logits
y equal `temp` exactly.  adj[b,i,j] = 1.0 iff temp[i,j] is among the 8
largest of row i, identical across the batch.

Sharding: row-parallel over N=2048 across 8 cores (256 rows/core, two
128-row chunks living side by side in one [128, 4096] SBUF tile).

v3 (vs the 23.9us single-queue baseline):
  - Both HWDGE queues (sync + scalar) stream inputs in parallel as full
    8KB-row descriptors, partition-halves per chunk.  A 64B warm-up DMA
    per queue primes the doorbell/descriptor pipeline so the first real
    transfer starts streaming sooner.
  - All compute stays on DVE (GpSimd tensor_scalar measured ~20x slower
    AND it starves concurrent DVE ops via SBUF contention).  The chunk0
    is_ge slots into DVE's idle window while chunk1 is still streaming.
  - out0 leaves on the scalar queue overlapped with chunk1's compute;
    out1 on the sync queue.  (Out descs run ~300GB/s, not a bottleneck.)

Host: concatenate 8 u8 slabs, cast to f32, broadcast over B=4.
"""

import sys

import numpy as np

if "/opt/trn_rl_repo" not in sys.path:
    sys.path.insert(0, "/opt/trn_rl_repo")

B, N, K = 4, 2048, 8
N_CORES = 8
ROWS = N // N_CORES  # 256 rows per core
P = 128  # SBUF partitions
HP = P // 2  # partition half per queue

# Hooks for a driving harness (test.py): extra kwargs for run_bass_kernel_spmd
# and the last BassKernelResults (exec_time_ns etc).
RUN_KWARGS: dict = {}
LAST_RESULT = None

_PROGRAM = None


def _build_program():
    import concourse.bass as bass
    import concourse.mybir as mybir

    class _LeanBass(bass.Bass):
        # Skip the barrier Bass.__init__ emits after const-AP registration:
        # this kernel never reads const APs, Sync's DGE table load precedes
        # its DMAs in program order, and the NRT entry pseudo-barrier already
        # orders the gpsimd sem-clears.  Saves ~1us of preamble.
        _skip_init_barrier = False

        def all_engine_barrier(self, **kw):
            if _LeanBass._skip_init_barrier:
                return
            return super().all_engine_barrier(**kw)

    _LeanBass._skip_init_barrier = True
    try:
        nc = _LeanBass(enable_partition_id=False, monotonic_sem_count=0)
    finally:
        _LeanBass._skip_init_barrier = False
    t_in = nc.declare_dram_parameter("t", [ROWS, N], mybir.dt.float32, isOutput=False)
    # u8 wire format for the 0/1 mask (lossless); host casts back to f32
    out = nc.declare_dram_parameter("out", [ROWS, N], mybir.dt.uint8, isOutput=True)

    with (
        nc.sbuf_tensor([P, 2 * N], mybir.dt.float32) as tile,
        nc.sbuf_tensor([P, 2 * N], mybir.dt.uint8) as mask,
        nc.sbuf_tensor([P, 16], mybir.dt.float32) as t8,
        nc.sbuf_tensor([P, 16], mybir.dt.float32) as scratch,
        # per-transfer in-DMA sems: transfers on different queues complete
        # out of order, so shared counting sems would race
        nc.semaphore("in_a0") as in_a0,
        nc.semaphore("in_b0") as in_b0,
        nc.semaphore("in_a1") as in_a1,
        nc.semaphore("in_b1") as in_b1,
        nc.semaphore("v_sem") as v_sem,
        nc.semaphore("out_sem") as out_sem,
        nc.semaphore("warm_sem") as warm_sem,
    ):
        # Warm-up: a 64B DMA per queue fills the doorbell->DGE->engine
        # pipeline while the real transfers are still being decoded.
        # (Nothing waits on warm_sem; the exit drain covers completion.)
        nc.sync.dma_start(out=scratch[0:1, :], in_=t_in[0:1, 0:16]).then_inc(
            warm_sem, 16
        )
        nc.scalar.dma_start(out=scratch[1:2, :], in_=t_in[1:2, 0:16]).then_inc(
            warm_sem, 16
        )

        # In-DMAs issued OUTSIDE the Block, right after each queue engine's
        # DGE-table preamble.  Full 8KB rows = one descriptor per partition;
        # partition-halves split each chunk across both queues.
        nc.sync.dma_start(out=tile[0:HP, 0:N], in_=t_in[0:HP, :]).then_inc(in_a0, 16)
        nc.scalar.dma_start(out=tile[HP:P, 0:N], in_=t_in[HP:P, :]).then_inc(in_b0, 16)
        nc.sync.dma_start(out=tile[0:HP, N : 2 * N], in_=t_in[P : P + HP, :]).then_inc(
            in_a1, 16
        )
        nc.scalar.dma_start(
            out=tile[HP:P, N : 2 * N], in_=t_in[P + HP : 2 * P, :]
        ).then_inc(in_b1, 16)

        # no SWDGE DMAs issued -> skip GpSimd's expensive dge_drain at exit
        with nc.Block(no_gpsimd_drain=True) as block:

            @block.vector
            def _(vector):
                # Flat MAX8 per chunk; the chunk0 is_ge runs while chunk1 is
                # still streaming in.  Sem self-hops guard the same-engine
                # RAW on t8 (tensor_scalar's scalar-ptr fetch races the
                # in-pipeline write of the preceding MAX8).
                vector.wait_ge(in_a0, 16)
                vector.wait_ge(in_b0, 16)
                vector.max(t8[:, 0:8], tile[:, 0:N]).then_inc(v_sem, 1)
                vector.wait_ge(v_sem, 1)
                vector.tensor_scalar(
                    mask[:, 0:N],
                    tile[:, 0:N],
                    t8[:, 7:8],
                    None,
                    mybir.AluOpType.is_ge,
                ).then_inc(v_sem, 1)
                vector.wait_ge(in_a1, 16)
                vector.wait_ge(in_b1, 16)
                vector.max(t8[:, 8:16], tile[:, N : 2 * N]).then_inc(v_sem, 1)
                vector.wait_ge(v_sem, 3)
                vector.tensor_scalar(
                    mask[:, N : 2 * N],
                    tile[:, N : 2 * N],
                    t8[:, 15:16],
                    None,
                    mybir.AluOpType.is_ge,
                ).then_inc(v_sem, 1)

            @block.scalar
            def _(scalar):
                # chunk0 mask out on the scalar queue, overlapped with the
                # chunk1 stream + compute
                scalar.wait_ge(v_sem, 2)
                scalar.dma_start(out=out[0:P, :], in_=mask[:, 0:N]).then_inc(
                    out_sem, 16
                )

            @block.sync
            def _(sync):
                sync.wait_ge(v_sem, 4)
                sync.dma_start(out=out[P : 2 * P, :], in_=mask[:, N : 2 * N]).then_inc(
                    out_sem, 16
                )
                sync.wait_ge(out_sem, 32)

    return nc


def kernel(**inputs: np.ndarray) -> np.ndarray:
    global _PROGRAM, LAST_RESULT
    from concourse.bass_utils import run_bass_kernel_spmd

    temp = np.ascontiguousarray(np.asarray(inputs["temp"], dtype=np.float32))
    assert temp.shape == (N, N)

    in_maps = [
        {"t": np.ascontiguousarray(temp[c * ROWS : (c + 1) * ROWS])}
        for c in range(N_CORES)
    ]

    res = None
    last_err = None
    for attempt in range(3):
        try:
            if _PROGRAM is None:
                _PROGRAM = _build_program()
            res = run_bass_kernel_spmd(
                _PROGRAM, in_maps, list(range(N_CORES)), **RUN_KWARGS
            )
            break
        except Exception as e:  # transient device wedges (e.g. NRT unrecoverable)
            last_err = e
            _PROGRAM = None
            if attempt == 2:
                raise
            import time

            time.sleep(10 * (attempt + 1))
            try:  # recreate the PJRT client, as a fresh process would
                import jax

                jax.clear_backends()
                jax.devices()
            except Exception:
                pass
    assert res is not None, last_err
    LAST_RESULT = res

    mask = np.concatenate([res.results[c]["out"] for c in range(N_CORES)], axis=0)
    mask = mask.astype(np.float32)
    return np.ascontiguousarray(np.broadcast_to(mask[None], (B, N, N)))


# revision 5
# speedup vs baseline: 2.8120x; 1.1093x over previous
"""Trainium2 Bass kernel for nn_DGG_StraightThrough.

The reference's pairwise-logit MLP is mathematically dead: softmax over the
singleton feature dim is identically 1, so log_p == 0 and the gumbel logits
y equal `temp` exactly.  adj[b,i,j] = 1.0 iff temp[i,j] is among the 8
largest of row i, identical across the batch.

Sharding: row-parallel over N=2048 across 8 cores (256 rows/core, two
128-row chunks living side by side in one [128, 4096] SBUF tile).

v4 (vs the 23.9us single-queue baseline):
  - Entry: the init-time dma_reset/sem_clear are redirected from GpSimd
    (whose ~3us instruction launch latency gates the NRT entry barrier)
    onto the Sync engine, which reaches the barrier in ~1us.
  - Both HWDGE queues (sync + scalar) co-stream chunk0's column halves
    first, then chunk1's, so chunk0 lands as early as possible and the
    DVE pipeline (max0, is_ge0 | max1, is_ge1) stays busy while chunk1
    streams.  All 8 cores share ~2.2TB/s of HBM, so queue count does not
    change aggregate BW -- only the landing ORDER matters.
  - All compute on DVE (GpSimd tensor_scalar measured ~20x slower and
    starves concurrent DVE ops).  out0 leaves on the scalar queue
    overlapped with chunk1 compute; out1 on the sync queue.

Host: concatenate 8 u8 slabs, cast to f32, broadcast over B=4.
"""

import sys

import numpy as np

if "/opt/trn_rl_repo" not in sys.path:
    sys.path.insert(0, "/opt/trn_rl_repo")

B, N, K = 4, 2048, 8
N_CORES = 8
ROWS = N // N_CORES  # 256 rows per core
P = 128  # SBUF partitions
H = N // 2  # column half per queue

# Hooks for a driving harness (test.py): extra kwargs for run_bass_kernel_spmd
# and the last BassKernelResults (exec_time_ns etc).
RUN_KWARGS: dict = {}
LAST_RESULT = None

_PROGRAM = None


def _build_program():
    import concourse.bass as bass
    import concourse.mybir as mybir

    class _LeanBass(bass.Bass):
        # Skip the barrier Bass.__init__ emits after const-AP registration:
        # this kernel never reads const APs, Sync's DGE table load precedes
        # its DMAs in program order, and the NRT entry pseudo-barrier already
        # orders the sem-clears.  Saves ~1us of preamble.
        _skip_init_barrier = False

        def all_engine_barrier(self, **kw):
            if _LeanBass._skip_init_barrier:
                return
            return super().all_engine_barrier(**kw)

    # Redirect the init-time sem-range drain + sem_clear from GpSimd to
    # Sync: they only need to precede the NRT pseudo-barrier, and GpSimd's
    # multi-us instruction launch latency otherwise gates the barrier
    # release for every engine.  One full-range drain suffices.
    def _reset_on_sync(self, semaphore_range=None):
        b = self.bass
        if getattr(b, "_lean_drained", False):
            return None
        b._lean_drained = True
        return b.sync.drain(semaphore_range=b._kernel_sem_range)

    def _clear_on_sync(self, rng):
        return self.bass.sync.sem_clear(rng)

    _LeanBass._skip_init_barrier = True
    orig_reset = bass.BassGpSimd.dma_reset
    orig_clear = bass.BassGpSimd.sem_clear
    bass.BassGpSimd.dma_reset = _reset_on_sync
    bass.BassGpSimd.sem_clear = _clear_on_sync
    try:
        nc = _LeanBass(enable_partition_id=False, monotonic_sem_count=0)
    finally:
        _LeanBass._skip_init_barrier = False
        bass.BassGpSimd.dma_reset = orig_reset
        bass.BassGpSimd.sem_clear = orig_clear
    t_in = nc.declare_dram_parameter("t", [ROWS, N], mybir.dt.float32, isOutput=False)
    # u8 wire format for the 0/1 mask (lossless); host casts back to f32
    out = nc.declare_dram_parameter("out", [ROWS, N], mybir.dt.uint8, isOutput=True)

    with (
        nc.sbuf_tensor([P, 2 * N], mybir.dt.float32) as tile,
        nc.sbuf_tensor([P, 2 * N], mybir.dt.uint8) as mask,
        nc.sbuf_tensor([P, 16], mybir.dt.float32) as t8,
        # per-transfer in-DMA sems: transfers on different queues complete
        # out of order, so shared counting sems would race
        nc.semaphore("in_a0") as in_a0,
        nc.semaphore("in_b0") as in_b0,
        nc.semaphore("in_a1") as in_a1,
        nc.semaphore("in_b1") as in_b1,
        nc.semaphore("v_sem") as v_sem,
        nc.semaphore("out_sem") as out_sem,
    ):
        # In-DMAs issued OUTSIDE the Block, right after each queue engine's
        # DGE-table preamble.  Chunk0's column halves go first on both
        # queues (4KB descriptors), then chunk1's, so chunk0 lands ~4us
        # before chunk1 and the DVE pipeline starts early.
        nc.sync.dma_start(out=tile[:, 0:H], in_=t_in[0:P, 0:H]).then_inc(in_a0, 16)
        nc.scalar.dma_start(out=tile[:, H:N], in_=t_in[0:P, H:N]).then_inc(in_b0, 16)
        nc.sync.dma_start(
            out=tile[:, N : N + H], in_=t_in[P : 2 * P, 0:H]
        ).then_inc(in_a1, 16)
        nc.scalar.dma_start(
            out=tile[:, N + H : 2 * N], in_=t_in[P : 2 * P, H:N]
        ).then_inc(in_b1, 16)

        # no SWDGE DMAs issued -> skip GpSimd's expensive dge_drain at exit
        with nc.Block(no_gpsimd_drain=True) as block:

            @block.vector
            def _(vector):
                # Flat MAX8 per chunk (both column halves land ~together on
                # the parallel queues, so a hierarchical merge buys nothing).
                # The chunk0 is_ge runs while chunk1 is still streaming in.
                # Sem self-hops guard the same-engine RAW on t8.
                vector.wait_ge(in_a0, 16)
                vector.wait_ge(in_b0, 16)
                vector.max(t8[:, 0:8], tile[:, 0:N]).then_inc(v_sem, 1)
                vector.wait_ge(v_sem, 1)
                vector.tensor_scalar(
                    mask[:, 0:N],
                    tile[:, 0:N],
                    t8[:, 7:8],
                    None,
                    mybir.AluOpType.is_ge,
                ).then_inc(v_sem, 1)
                vector.wait_ge(in_a1, 16)
                vector.wait_ge(in_b1, 16)
                vector.max(t8[:, 8:16], tile[:, N : 2 * N]).then_inc(v_sem, 1)
                vector.wait_ge(v_sem, 3)
                vector.tensor_scalar(
                    mask[:, N : 2 * N],
                    tile[:, N : 2 * N],
                    t8[:, 15:16],
                    None,
                    mybir.AluOpType.is_ge,
                ).then_inc(v_sem, 1)

            @block.scalar
            def _(scalar):
                # chunk0 mask out on the scalar queue, overlapped with the
                # chunk1 stream + compute
                scalar.wait_ge(v_sem, 2)
                scalar.dma_start(out=out[0:P, :], in_=mask[:, 0:N]).then_inc(
                    out_sem, 16
                )

            @block.sync
            def _(sync):
                sync.wait_ge(v_sem, 4)
                sync.dma_start(out=out[P : 2 * P, :], in_=mask[:, N : 2 * N]).then_inc(
                    out_sem, 16
                )
                sync.wait_ge(out_sem, 32)

    return nc


def kernel(**inputs: np.ndarray) -> np.ndarray:
    global _PROGRAM, LAST_RESULT
    from concourse.bass_utils import run_bass_kernel_spmd

    temp = np.ascontiguousarray(np.asarray(inputs["temp"], dtype=np.float32))
    assert temp.shape == (N, N)

    in_maps = [
        {"t": np.ascontiguousarray(temp[c * ROWS : (c + 1) * ROWS])}
        for c in range(N_CORES)
    ]

    res = None
    last_err = None
    for attempt in range(3):
        try:
            if _PROGRAM is None:
                _PROGRAM = _build_program()
            res = run_bass_kernel_spmd(
                _PROGRAM, in_maps, list(range(N_CORES)), **RUN_KWARGS
            )
            break
        except Exception as e:  # transient device wedges (e.g. NRT unrecoverable)
            last_err = e
            _PROGRAM = None
            if attempt == 2:
                raise
            import time

            time.sleep(10 * (attempt + 1))
            try:  # recreate the PJRT client, as a fresh process would
                import jax

                jax.clear_backends()
                jax.devices()
            except Exception:
                pass
    assert res is not None, last_err
    LAST_RESULT = res

    mask = np.concatenate([res.results[c]["out"] for c in range(N_CORES)], axis=0)
    mask = mask.astype(np.float32)
    return np.ascontiguousarray(np.broadcast_to(mask[None], (B, N, N)))
